# revision 41
# baseline (speedup 1.0000x reference)
"""MQA attention (LN + QKV proj + RoPE + causal attn + out-proj) on 8 trn2 cores.

Sharding: tensor-parallel over heads (2 heads/core, Wq cols + Wo rows), KV
replicated (single KV head), out-proj produces per-core partial sums that the
host reduces.

v4 per-core dataflow:
  Host ships BOTH x (natural, for LN stats) and xT (transposed, for matmul
  rhs) — no on-device activation transposes (DMA-xbar transposes serialize
  against all other DMA traffic, ~8.5us each; PE transposes burn the
  critical engine).
  LayerNorm is folded into the projections:
      proj(LN(x)) = (W^T x - csum * mu_t) * rstd_t
  with csum = W^T 1 precomputed on host, and mu/rstd per token applied at
  PSUM-evict time (rstd folded into the RoPE cos/sin for q,k).
  rstd comes from a Newton rsqrt on DVE (no ACT sqrt -> single exp table).
  Per-token stats cross from partition-layout to row-layout via a tiny
  DRAM bounce (gpsimd SWDGE), as does the softmax-sum reciprocal.
  Attention: S^T = k @ q^T causal-blocked -> exp on ScalarE -> AV + row
  sums via ones-matmul -> normalize -> out-proj partial -> host sum.
  Emission pipelines b0-attention with b1-projections and out-proj with
  attention to keep the PE dense (HAM stays warm).
"""

import sys

if "/opt/trn_rl_repo" not in sys.path:
    sys.path.insert(0, "/opt/trn_rl_repo")

import ml_dtypes
import numpy as np

import concourse.bass as bass
import concourse.tile as tile
from concourse import bacc, mybir
from concourse.masks import make_identity

F32 = mybir.dt.float32
DT = mybir.dt.bfloat16  # matmul operand storage dtype
DT_NP = ml_dtypes.bfloat16

B, N, DIM, DH, HEADS = 2, 2048, 2048, 128, 16
H_LOCAL = 2  # heads per core
N_CORES = 8
KT = DIM // 128  # k-tiles over the model dim
TT = N // 128  # token tiles per batch
CHUNK = 512  # token chunk for projection phase
NCH = N // CHUNK  # chunks per batch
QG = 512  # q-group width in attention
NQG = N // QG
SCALE = float(DH) ** -0.5
EPS = 1e-5
NEG = -1e30
WARM_MMS = 48


def build_nc(repeat=1):
    nc = bacc.Bacc(None, target_bir_lowering=False, debug=False)

    x_d = nc.dram_tensor("x_in", [B, N, DIM], DT, kind="ExternalInput")
    xt_d = nc.dram_tensor("xT", [128, B, NCH, KT, CHUNK], DT, kind="ExternalInput")
    wq_d = nc.dram_tensor("wq", [128, KT, H_LOCAL * DH], DT, kind="ExternalInput")
    wk_d = nc.dram_tensor("wk", [128, KT, DH], DT, kind="ExternalInput")
    wv_d = nc.dram_tensor("wv", [128, KT, DH], DT, kind="ExternalInput")
    wo_d = nc.dram_tensor("wo", [128, H_LOCAL, DIM], DT, kind="ExternalInput")
    csr_d = nc.dram_tensor("csr", [1, 4, 128], DT, kind="ExternalInput")
    cos_d = nc.dram_tensor("cosT", [DH, N], DT, kind="ExternalInput")
    sin_d = nc.dram_tensor("sinT", [DH, N], DT, kind="ExternalInput")
    msk_d = nc.dram_tensor("mask", [128, 128], DT, kind="ExternalInput")
    out_d = nc.dram_tensor("out_partial", [B, N, DIM], DT, kind="ExternalOutput")

    with tile.TileContext(nc) as tc:
        with (
            tc.tile_pool(name="const", bufs=1) as const,
            tc.tile_pool(name="xp", bufs=8) as xp,
            tc.tile_pool(name="xtp", bufs=2) as xtp,
            tc.tile_pool(name="store", bufs=1) as store,
            tc.tile_pool(name="small", bufs=6) as small,
            tc.tile_pool(name="rope", bufs=3) as ropep,
            tc.tile_pool(name="bc", bufs=3) as bcp,
            tc.tile_pool(name="ep", bufs=4) as ep,
            tc.tile_pool(name="sm", bufs=2) as sm,
            tc.tile_pool(name="rw", bufs=3) as rwp,
            tc.tile_pool(name="vb", bufs=2) as vb,
            tc.tile_pool(name="op", bufs=3) as op,
            tc.tile_pool(name="dr", bufs=3, space="DRAM") as drp,
            tc.tile_pool(name="ps", bufs=1, space="PSUM") as ps,
        ):
            # --- first chunk's inputs before the weights so LN starts early ---
            pre_x = {}

            def load_x(b, cg):
                key = (b, cg)
                if key in pre_x:
                    return pre_x.pop(key)
                tiles = []
                c0 = cg * CHUNK
                for t in range(CHUNK // 128):
                    tok0 = c0 + t * 128
                    x_t = xp.tile([128, DIM], DT, tag="x", name=f"x_{b}_{cg}_{t}")
                    nc.gpsimd.dma_start(x_t[:], x_d[b, tok0 : tok0 + 128, :])
                    tiles.append(x_t)
                return tiles

            pre_xt = {}

            def load_xt(b, cg):
                key = (b, cg)
                if key in pre_xt:
                    return pre_xt.pop(key)
                xtc = xtp.tile([128, KT, CHUNK], DT, tag="xtc", name=f"xt_{b}_{cg}")
                nc.gpsimd.dma_start(xtc[:], xt_d[:, b, cg])
                return xtc

            pre_xt[(0, 0)] = load_xt(0, 0)
            pre_x[(0, 0)] = load_x(0, 0)

            # --- constants ---
            wq_sb = const.tile([128, KT, H_LOCAL * DH], DT)
            nc.sync.dma_start(wq_sb[:], wq_d[:])
            wk_sb = const.tile([128, KT, DH], DT)
            nc.sync.dma_start(wk_sb[:], wk_d[:])
            wv_sb = const.tile([128, KT, DH], DT)
            nc.sync.dma_start(wv_sb[:], wv_d[:])
            cos_sb = const.tile([DH, N], DT)
            nc.sync.dma_start(cos_sb[:], cos_d[:])
            sin_sb = const.tile([DH, N], DT)
            nc.sync.dma_start(sin_sb[:], sin_d[:])
            csr_sb = const.tile([1, 4, 128], DT)
            nc.sync.dma_start(csr_sb[:], csr_d[:])
            ones_mm = const.tile([128, 1], DT)
            nc.vector.memset(ones_mm, 1.0)
            ident = const.tile([128, 128], DT)
            make_identity(nc, ident)

            # --- PE warmup: wake the HAM clock gate before real work.
            # One accumulation group: back-to-back MMs, no inter-MM sems.
            warm = const.tile([128, 512], DT)
            nc.vector.memset(warm, 0.0)
            wps = ps.tile([128, 512], F32, tag="s", bufs=2)
            for i in range(WARM_MMS):
                nc.tensor.matmul(
                    wps[:], warm[:, 0:128], warm[:],
                    start=(i == 0), stop=(i == WARM_MMS - 1),
                )

            # needed later than the above: out-proj weights + causal mask
            msk_sb = const.tile([128, 128], DT)
            nc.sync.dma_start(msk_sb[:], msk_d[:])
            wo_sb = const.tile([128, H_LOCAL, DIM], DT)
            nc.sync.dma_start(wo_sb[:], wo_d[:])

            # --- persistent activations (per batch for loose cross-phase deps) ---
            qT_b = [
                store.tile([DH, H_LOCAL, N], DT, tag=f"qT{b}", name=f"qT{b}")
                for b in range(B)
            ]
            kT_b = [
                store.tile([DH, N], DT, tag=f"kT{b}", name=f"kT{b}") for b in range(B)
            ]
            v_b = [
                store.tile([128, TT, DH], DT, tag=f"v{b}", name=f"v{b}")
                for b in range(B)
            ]
            aoT_b = [
                store.tile([DH, H_LOCAL, N], DT, tag=f"aoT{b}", name=f"aoT{b}")
                for b in range(B)
            ]

            ln_state = {}

            def ln_stats_a(b, cg):
                """First half of LN stats (tiles 0-1) — split so the DVE queue
                never sees one long stats lump that delays PSUM evictions."""
                xts = load_x(b, cg)
                mr = small.tile([128, 4, 2], F32, tag="mr")
                for t in (0, 1):
                    stats = small.tile([128, 4, 6], F32, tag="stats")
                    for i in range(4):
                        nc.vector.bn_stats(
                            out=stats[:, i, :], in_=xts[t][:, i * 512 : (i + 1) * 512]
                        )
                    nc.vector.bn_aggr(out=mr[:, t, :], in_=stats[:])
                ln_state[(b, cg)] = (xts, mr)

            def ln_stats_b(b, cg):
                """Second half: stats tiles 2-3, rsqrt, row bounce, broadcast.

                Returns (mu_bc, rstd_bc, m1_bc) row-broadcast views [128, CHUNK].
                """
                xts, mr = ln_state.pop((b, cg))
                for t in (2, 3):
                    stats = small.tile([128, 4, 6], F32, tag="stats")
                    for i in range(4):
                        nc.vector.bn_stats(
                            out=stats[:, i, :], in_=xts[t][:, i * 512 : (i + 1) * 512]
                        )
                    nc.vector.bn_aggr(out=mr[:, t, :], in_=stats[:])
                # Newton rsqrt: y = rsqrt(var+eps), y0 = 1.25-0.25v, 4 iters
                mp = small.tile([128, 2, 4], F32, tag="mp")
                ve = small.tile([128, 4], F32, tag="ve")
                nc.vector.tensor_scalar(
                    out=ve[:], in0=mr[:, :, 1], scalar1=EPS, scalar2=None,
                    op0=mybir.AluOpType.add,
                )
                y = small.tile([128, 4], F32, tag="y")
                nc.vector.tensor_scalar(
                    out=y[:], in0=ve[:], scalar1=-0.25, scalar2=1.25,
                    op0=mybir.AluOpType.mult, op1=mybir.AluOpType.add,
                )
                a = small.tile([128, 4], F32, tag="a")
                for it in range(4):
                    nc.vector.tensor_mul(a[:], y[:], y[:])
                    nc.vector.tensor_mul(a[:], a[:], ve[:])
                    nc.vector.tensor_scalar(
                        out=a[:], in0=a[:], scalar1=-0.5, scalar2=1.5,
                        op0=mybir.AluOpType.mult, op1=mybir.AluOpType.add,
                    )
                    dst = mp[:, 1, :] if it == 3 else y[:]
                    nc.vector.tensor_mul(dst, y[:], a[:])
                nc.vector.tensor_copy(mp[:, 0, :], mr[:, :, 0])
                # partition-layout -> row-layout via DRAM bounce (SWDGE):
                # mu stays a row (consumed by rank-1 correction matmuls on
                # the PE); only rstd needs a partition broadcast.
                md = drp.tile([128, 2, 4], F32, tag="md")
                nc.gpsimd.dma_start(md[:], mp[:])
                row2 = rwp.tile([1, 2 * CHUNK], DT, tag="row2", name=f"row2_{b}_{cg}")
                nc.gpsimd.dma_start(row2[:], md.rearrange("p c t -> c t p"))
                rsbc = bcp.tile([128, CHUNK], DT, tag="rsbc", name=f"rsbc_{b}_{cg}")
                nc.gpsimd.partition_broadcast(rsbc[:], row2[:, CHUNK : 2 * CHUNK])
                return row2, rsbc  # mu row [1,0:CHUNK] of row2, rstd bcast

            def rope_evict(dst, src_ps, cosp, sinp):
                # dst = ROPE(src) * rstd  (rstd folded into cosp/sinp;
                # the -cs*mu correction was accumulated on the PE)
                pc = ropep.tile([DH, CHUNK], DT, tag="pc")
                nc.vector.tensor_copy(pc[:], src_ps[:])
                rot = ropep.tile([DH, CHUNK], DT, tag="rot")
                nc.scalar.copy(rot[0:64, :], pc[64:128, :])
                nc.scalar.copy(rot[64:128, :], pc[0:64, :])
                tmp = ropep.tile([DH, CHUNK], DT, tag="tmp")
                nc.vector.tensor_mul(tmp[:], pc[:], cosp[:])
                nc.vector.tensor_mul(rot[:], rot[:], sinp[:])
                nc.vector.tensor_add(dst, tmp[:], rot[:])

            def chunk_proj(b, cg, lnr):
                c0 = cg * CHUNK
                row2, rstd_bc = lnr
                xtc = load_xt(b, cg)
                # rstd folded into the rope multipliers (shared by q0,q1,k)
                cosp = ropep.tile([DH, CHUNK], DT, tag="cosp")
                nc.vector.tensor_mul(cosp[:], cos_sb[:, c0 : c0 + CHUNK], rstd_bc)
                sinp = ropep.tile([DH, CHUNK], DT, tag="sinp")
                nc.vector.tensor_mul(sinp[:], sin_sb[:, c0 : c0 + CHUNK], rstd_bc)
                # wave 1: q heads
                qt0 = ps.tile([DH, CHUNK], F32, tag="acc", bufs=3)
                qt1 = ps.tile([DH, CHUNK], F32, tag="acc", bufs=3)
                for kt in range(KT):
                    rhs = xtc[:, kt, :]
                    nc.tensor.matmul(
                        qt0[:], wq_sb[:, kt, 0:128], rhs,
                        start=(kt == 0), stop=False,
                    )
                    nc.tensor.matmul(
                        qt1[:], wq_sb[:, kt, 128:256], rhs,
                        start=(kt == 0), stop=False,
                    )
                nc.tensor.matmul(
                    qt0[:], csr_sb[:, 0, :], row2[:, 0:CHUNK],
                    start=False, stop=True,
                )
                nc.tensor.matmul(
                    qt1[:], csr_sb[:, 1, :], row2[:, 0:CHUNK],
                    start=False, stop=True,
                )
                rope_evict(qT_b[b][:, 0, c0 : c0 + CHUNK], qt0, cosp, sinp)
                rope_evict(qT_b[b][:, 1, c0 : c0 + CHUNK], qt1, cosp, sinp)
                # wave 2: k, v
                ktp = ps.tile([DH, CHUNK], F32, tag="acc", bufs=3)
                vtp = ps.tile([DH, CHUNK], F32, tag="acc", bufs=3)
                for kt in range(KT):
                    rhs = xtc[:, kt, :]
                    nc.tensor.matmul(
                        ktp[:], wk_sb[:, kt, :], rhs,
                        start=(kt == 0), stop=False,
                    )
                    nc.tensor.matmul(
                        vtp[:], wv_sb[:, kt, :], rhs,
                        start=(kt == 0), stop=False,
                    )
                nc.tensor.matmul(
                    ktp[:], csr_sb[:, 2, :], row2[:, 0:CHUNK],
                    start=False, stop=True,
                )
                nc.tensor.matmul(
                    vtp[:], csr_sb[:, 3, :], row2[:, 0:CHUNK],
                    start=False, stop=True,
                )
                rope_evict(kT_b[b][:, c0 : c0 + CHUNK], ktp, cosp, sinp)
                # v: scale by rstd, then PE-transpose to natural [tok, dh]
                vT = vb.tile([DH, CHUNK], DT, tag="vT")
                nc.vector.tensor_mul(vT[:], vtp[:], rstd_bc)
                vn_ps = ps.tile([128, 512], F32, tag="s", bufs=2)
                for tv in range(4):
                    nc.tensor.matmul(
                        vn_ps[:, tv * 128 : (tv + 1) * 128],
                        vT[:, tv * 128 : (tv + 1) * 128],
                        ident[:],
                    )
                nc.scalar.copy(
                    v_b[b][:, cg * 4 : (cg + 1) * 4, :],
                    vn_ps[:].rearrange("p (t d) -> p t d", t=4),
                )

            def attn_group(b, h, qg):
                q0 = qg * QG
                nkt = (qg + 1) * (QG // 128)
                avT = ps.tile([DH, QG], F32, tag="av", bufs=2)
                sums = ps.tile([1, QG], F32, tag="sums", bufs=1)
                # software-pipelined emission: S(kt+1) is emitted BEFORE
                # AV(kt) so the PE FIFO never waits for exp(kt) — the exp
                # hides behind the next S matmul.
                pend = None
                for kt in range(nkt):
                    off = max(0, kt * 128 - q0)
                    diag = kt * 128 >= q0
                    st = ps.tile([128, QG], F32, tag="s", bufs=2)
                    nc.tensor.matmul(
                        st[:, off:],
                        kT_b[b][:, kt * 128 : (kt + 1) * 128],
                        qT_b[b][:, h, q0 + off : q0 + QG],
                        start=True,
                        stop=not diag,
                    )
                    if diag:  # causal mask accumulated on the PE itself
                        nc.tensor.matmul(
                            st[:, off : off + 128],
                            ident[:],
                            msk_sb[:],
                            start=False,
                            stop=True,
                        )
                    et = ep.tile([128, QG], DT, tag="et")
                    nc.scalar.activation(
                        out=et[:, off:],
                        in_=st[:, off:],
                        func=mybir.ActivationFunctionType.Exp,
                        scale=SCALE,
                    )
                    if pend is not None:
                        pkt, poff, pet = pend
                        nc.tensor.matmul(
                            avT[:, poff:],
                            v_b[b][:, pkt, :],
                            pet[:, poff:],
                            start=(pkt == 0),
                            stop=False,
                        )
                        nc.tensor.matmul(
                            sums[:, poff:],
                            ones_mm[:],
                            pet[:, poff:],
                            start=(pkt == 0),
                            stop=False,
                        )
                    pend = (kt, off, et)
                pkt, poff, pet = pend
                nc.tensor.matmul(
                    avT[:, poff:],
                    v_b[b][:, pkt, :],
                    pet[:, poff:],
                    start=(pkt == 0),
                    stop=True,
                )
                nc.tensor.matmul(
                    sums[:, poff:],
                    ones_mm[:],
                    pet[:, poff:],
                    start=(pkt == 0),
                    stop=True,
                )
                # reciprocal on a [128, 4] reshape via DRAM bounce
                # (DVE recip is FD-serial: [1,512] costs ~2.3us, [128,4] ~0.1us)
                ss = sm.tile([1, QG], F32, tag="ss")
                nc.vector.tensor_copy(ss[:], sums[:])
                dr = drp.tile([1, QG], F32, tag="dr")
                nc.gpsimd.dma_start(dr[:], ss[:])
                rt = sm.tile([128, 4], F32, tag="rt")
                nc.gpsimd.dma_start(
                    rt[:], dr.rearrange("o (p f) -> (o p) f", p=128)
                )
                nc.vector.reciprocal(out=rt[:], in_=rt[:])
                dr2 = drp.tile([1, QG], F32, tag="dr2")
                nc.gpsimd.dma_start(
                    dr2.rearrange("o (p f) -> (o p) f", p=128), rt[:]
                )
                rr = sm.tile([1, QG], F32, tag="rr")
                nc.gpsimd.dma_start(rr[:], dr2[:])
                rbc = sm.tile([128, QG], F32, tag="rbc")
                nc.gpsimd.partition_broadcast(rbc[:], rr[:])
                nc.vector.tensor_mul(
                    aoT_b[b][:, h, q0 : q0 + QG], avT[:], rbc[:]
                )

            def outproj_group(b, tt):
                ot = op.tile([128, DIM], DT, tag="ot")
                for dg in range(4):
                    opp = ps.tile([128, 512], F32, tag="acc", bufs=3)
                    for h in range(H_LOCAL):
                        nc.tensor.matmul(
                            opp[:],
                            aoT_b[b][:, h, tt * 128 : (tt + 1) * 128],
                            wo_sb[:, h, dg * 512 : (dg + 1) * 512],
                            start=(h == 0),
                            stop=(h == H_LOCAL - 1),
                        )
                    if dg % 2 == 0:
                        nc.scalar.copy(ot[:, dg * 512 : (dg + 1) * 512], opp[:])
                    else:
                        nc.vector.tensor_copy(ot[:, dg * 512 : (dg + 1) * 512], opp[:])
                nc.sync.dma_start(out_d[b, tt * 128 : (tt + 1) * 128, :], ot[:])

            for _rep in range(repeat):
                # LN stats run one full round ahead of the projection that
                # consumes them: the rank-1 correction matmul (and thus the
                # PSUM-group close + slot release) needs the mu row, so the
                # stats chain must never be the thing the PE waits on.
                lnr = {}
                ln_stats_a(0, 0)
                lnr[(0, 0)] = ln_stats_b(0, 0)
                ln_stats_a(0, 1)
                lnr[(0, 1)] = ln_stats_b(0, 1)
                pre_xt[(0, 1)] = load_xt(0, 1)
                chunk_proj(0, 0, lnr.pop((0, 0)))
                ln_stats_a(0, 2)
                lnr[(0, 2)] = ln_stats_b(0, 2)
                pre_xt[(0, 2)] = load_xt(0, 2)
                chunk_proj(0, 1, lnr.pop((0, 1)))
                ln_stats_a(0, 3)
                pre_xt[(0, 3)] = load_xt(0, 3)
                chunk_proj(0, 2, lnr.pop((0, 2)))
                lnr[(0, 3)] = ln_stats_b(0, 3)
                ln_stats_a(1, 0)
                pre_xt[(1, 0)] = load_xt(1, 0)
                chunk_proj(0, 3, lnr.pop((0, 3)))
                lnr[(1, 0)] = ln_stats_b(1, 0)
                # phase 2: b0 attention + b1 projections + b0 out-proj (lagged)
                for qg in range(NQG):
                    if qg < NQG - 1:
                        ln_stats_a(1, qg + 1)
                    if qg > 0:
                        for tt in range(4 * (qg - 1), 4 * qg):
                            outproj_group(0, tt)
                    attn_group(0, 0, qg)
                    if qg < NQG - 1:
                        pre_xt[(1, qg + 1)] = load_xt(1, qg + 1)
                    attn_group(0, 1, qg)
                    chunk_proj(1, qg, lnr.pop((1, qg)))
                    if qg < NQG - 1:
                        lnr[(1, qg + 1)] = ln_stats_b(1, qg + 1)
                # phase 3: b1 attention + remaining out-proj (still lagged)
                for qg in range(NQG):
                    if qg == 0:
                        for tt in range(12, 16):
                            outproj_group(0, tt)
                    else:
                        for tt in range(4 * (qg - 1), 4 * qg):
                            outproj_group(1, tt)
                    attn_group(1, 0, qg)
                    attn_group(1, 1, qg)
                for tt in range(12, 16):
                    outproj_group(1, tt)

    nc.compile()
    return nc


def make_in_maps(x, gamma, Wq, Wkv, Wo):
    xbf = np.asarray(x, dtype=np.float32).astype(DT_NP)
    x_nat = np.ascontiguousarray(xbf)
    # xT[p, b, cg, kt, t] = x[b, cg*CHUNK + t, kt*128 + p]
    xT = np.ascontiguousarray(
        xbf.reshape(B, NCH, CHUNK, KT, 128).transpose(4, 0, 1, 3, 2)
    )
    g = np.asarray(gamma, dtype=np.float32)
    Wq = np.asarray(Wq, dtype=np.float32) * g[:, None]
    Wkv = np.asarray(Wkv, dtype=np.float32) * g[:, None]
    Wo = np.asarray(Wo, dtype=np.float32)

    t = np.arange(N, dtype=np.float64)
    inv = 1.0 / (10000.0 ** (np.arange(0, DH, 2, dtype=np.float64) / DH))  # [64]
    fr = np.outer(inv, t)  # [d, t]
    cosT = np.concatenate([np.cos(fr), np.cos(fr)], 0).astype(DT_NP)
    sinT = np.concatenate([-np.sin(fr), np.sin(fr)], 0).astype(DT_NP)
    mask = np.where(
        np.arange(128)[:, None] > np.arange(128)[None, :], NEG, 0.0
    ).astype(DT_NP)

    def pt(w):  # [DIM, M] -> [128, KT, M] partition-major
        return np.ascontiguousarray(
            w.reshape(KT, 128, -1).transpose(1, 0, 2).astype(DT_NP)
        )

    Wk = Wkv[:, :DH]
    Wv = Wkv[:, DH:]
    # negated column sums for the rank-1 LN-fold correction matmul
    csk = Wk.astype(DT_NP).astype(np.float32).sum(0)
    csv = Wv.astype(DT_NP).astype(np.float32).sum(0)
    maps = []
    for c in range(N_CORES):
        Wq_c = Wq[:, c * H_LOCAL * DH : (c + 1) * H_LOCAL * DH]
        wq_c = pt(Wq_c)
        csq_c = Wq_c.astype(DT_NP).astype(np.float32).sum(0).reshape(H_LOCAL, DH)
        csr_c = np.ascontiguousarray(
            (-np.stack([csq_c[0], csq_c[1], csk, csv])[None]).astype(DT_NP)
        )
        wo_c = np.ascontiguousarray(
            Wo[c * H_LOCAL * DH : (c + 1) * H_LOCAL * DH]
            .reshape(H_LOCAL, DH, DIM)
            .transpose(1, 0, 2)
            .astype(DT_NP)
        )
        maps.append(
            {
                "x_in": x_nat,
                "xT": xT,
                "wq": wq_c,
                "wk": pt(Wk),
                "wv": pt(Wv),
                "wo": wo_c,
                "csr": csr_c,
                "cosT": cosT,
                "sinT": sinT,
                "mask": mask,
            }
        )
    return maps


_NC_CACHE = {}


def get_nc(repeat=1, phase=4):
    key = (repeat,)
    if key not in _NC_CACHE:
        _NC_CACHE[key] = build_nc(repeat)
    return _NC_CACHE[key]


def kernel(x, gamma, Wq, Wkv, Wo, _trace=False, _repeat=1):
    from concourse import bass_utils

    nc = get_nc(_repeat)
    in_maps = make_in_maps(x, gamma, Wq, Wkv, Wo)
    res = bass_utils.run_bass_kernel_spmd(
        nc, in_maps, core_ids=list(range(N_CORES)), trace=_trace
    )
    out = np.zeros((B, N, DIM), dtype=np.float32)
    for r in res.results:
        out += np.asarray(r["out_partial"], dtype=np.float32)
    if _trace:
        kernel.last_results = res
    return out


# revision 43
# speedup vs baseline: 1.1814x; 1.1814x over previous
"""MQA attention (LN + QKV proj + RoPE + causal attn + out-proj) on 8 trn2 cores.

Sharding: tensor-parallel over heads (2 heads/core, Wq cols + Wo rows), KV
replicated (single KV head), out-proj produces per-core partial sums that the
host reduces.

v4 per-core dataflow:
  Host ships BOTH x (natural, for LN stats) and xT (transposed, for matmul
  rhs) — no on-device activation transposes (DMA-xbar transposes serialize
  against all other DMA traffic, ~8.5us each; PE transposes burn the
  critical engine).
  LayerNorm is folded into the projections:
      proj(LN(x)) = (W^T x - csum * mu_t) * rstd_t
  with csum = W^T 1 precomputed on host, and mu/rstd per token applied at
  PSUM-evict time (rstd folded into the RoPE cos/sin for q,k).
  rstd comes from a Newton rsqrt on DVE (no ACT sqrt -> single exp table).
  Per-token stats cross from partition-layout to row-layout via a tiny
  DRAM bounce (gpsimd SWDGE), as does the softmax-sum reciprocal.
  Attention: S^T = k @ q^T causal-blocked -> exp on ScalarE -> AV + row
  sums via ones-matmul -> normalize -> out-proj partial -> host sum.
  Emission pipelines b0-attention with b1-projections and out-proj with
  attention to keep the PE dense (HAM stays warm).
"""

import sys

if "/opt/trn_rl_repo" not in sys.path:
    sys.path.insert(0, "/opt/trn_rl_repo")

import ml_dtypes
import numpy as np

import concourse.bass as bass
import concourse.tile as tile
from concourse import bacc, mybir
from concourse.masks import make_identity

F32 = mybir.dt.float32
DT = mybir.dt.bfloat16  # matmul operand storage dtype
DT_NP = ml_dtypes.bfloat16

B, N, DIM, DH, HEADS = 2, 2048, 2048, 128, 16
H_LOCAL = 2  # heads per core
N_CORES = 8
KT = DIM // 128  # k-tiles over the model dim
TT = N // 128  # token tiles per batch
CHUNK = 512  # token chunk for projection phase
NCH = N // CHUNK  # chunks per batch
QG = 512  # q-group width in attention
NQG = N // QG
SCALE = float(DH) ** -0.5
EPS = 1e-5
NEG = -1e30
WARM_MMS = 48


def build_nc(repeat=1):
    nc = bacc.Bacc(None, target_bir_lowering=False, debug=False)

    x_d = nc.dram_tensor("x_in", [B, N, DIM], DT, kind="ExternalInput")
    xt_d = nc.dram_tensor("xT", [128, B, NCH, KT, CHUNK], DT, kind="ExternalInput")
    wq_d = nc.dram_tensor("wq", [128, KT, H_LOCAL * DH], DT, kind="ExternalInput")
    wk_d = nc.dram_tensor("wk", [128, KT, DH], DT, kind="ExternalInput")
    wv_d = nc.dram_tensor("wv", [128, KT, DH], DT, kind="ExternalInput")
    wo_d = nc.dram_tensor("wo", [128, H_LOCAL, DIM], DT, kind="ExternalInput")
    csr_d = nc.dram_tensor("csr", [1, 4, 128], DT, kind="ExternalInput")
    cos_d = nc.dram_tensor("cosT", [DH, N], DT, kind="ExternalInput")
    sin_d = nc.dram_tensor("sinT", [DH, N], DT, kind="ExternalInput")
    msk_d = nc.dram_tensor("mask", [128, 128], DT, kind="ExternalInput")
    out_d = nc.dram_tensor("out_partial", [B, N, DIM], DT, kind="ExternalOutput")

    with tile.TileContext(nc) as tc:
        with (
            tc.tile_pool(name="const", bufs=1) as const,
            tc.tile_pool(name="xp", bufs=8) as xp,
            tc.tile_pool(name="xtp", bufs=2) as xtp,
            tc.tile_pool(name="store", bufs=1) as store,
            tc.tile_pool(name="small", bufs=6) as small,
            tc.tile_pool(name="rope", bufs=2) as ropep,
            tc.tile_pool(name="bc", bufs=3) as bcp,
            tc.tile_pool(name="ep", bufs=4) as ep,
            tc.tile_pool(name="sm", bufs=2) as sm,
            tc.tile_pool(name="rw", bufs=3) as rwp,
            tc.tile_pool(name="vb", bufs=2) as vb,
            tc.tile_pool(name="op", bufs=3) as op,
            tc.tile_pool(name="dr", bufs=3, space="DRAM") as drp,
            tc.tile_pool(name="ps", bufs=1, space="PSUM") as ps,
        ):
            # --- first chunk's inputs before the weights so LN starts early ---
            pre_x = {}

            def load_x(b, cg):
                key = (b, cg)
                if key in pre_x:
                    return pre_x.pop(key)
                tiles = []
                c0 = cg * CHUNK
                for t in range(CHUNK // 128):
                    tok0 = c0 + t * 128
                    x_t = xp.tile([128, DIM], DT, tag="x", name=f"x_{b}_{cg}_{t}")
                    nc.gpsimd.dma_start(x_t[:], x_d[b, tok0 : tok0 + 128, :])
                    tiles.append(x_t)
                return tiles

            pre_xt = {}

            def load_xt(b, cg):
                key = (b, cg)
                if key in pre_xt:
                    return pre_xt.pop(key)
                xtc = xtp.tile([128, KT, CHUNK], DT, tag="xtc", name=f"xt_{b}_{cg}")
                nc.gpsimd.dma_start(xtc[:], xt_d[:, b, cg])
                return xtc

            pre_xt[(0, 0)] = load_xt(0, 0)
            pre_x[(0, 0)] = load_x(0, 0)

            # --- constants ---
            wq_sb = const.tile([128, KT, H_LOCAL * DH], DT)
            nc.sync.dma_start(wq_sb[:], wq_d[:])
            wk_sb = const.tile([128, KT, DH], DT)
            nc.sync.dma_start(wk_sb[:], wk_d[:])
            wv_sb = const.tile([128, KT, DH], DT)
            nc.sync.dma_start(wv_sb[:], wv_d[:])
            cos_sb = const.tile([DH, N], DT)
            nc.sync.dma_start(cos_sb[:], cos_d[:])
            sin_sb = const.tile([DH, N], DT)
            nc.sync.dma_start(sin_sb[:], sin_d[:])
            csr_sb = const.tile([1, 4, 128], DT)
            nc.sync.dma_start(csr_sb[:], csr_d[:])
            ones_mm = const.tile([128, 1], DT)
            nc.vector.memset(ones_mm, 1.0)
            ident = const.tile([128, 128], DT)
            make_identity(nc, ident)

            # --- PE warmup: wake the HAM clock gate before real work.
            # One accumulation group: back-to-back MMs, no inter-MM sems.
            warm = const.tile([128, 512], DT)
            nc.vector.memset(warm, 0.0)
            wps = ps.tile([128, 512], F32, tag="s", bufs=2)
            for i in range(WARM_MMS):
                nc.tensor.matmul(
                    wps[:], warm[:, 0:128], warm[:],
                    start=(i == 0), stop=(i == WARM_MMS - 1),
                )

            # needed later than the above: out-proj weights + causal mask
            msk_sb = const.tile([128, 128], DT)
            nc.sync.dma_start(msk_sb[:], msk_d[:])
            wo_sb = const.tile([128, H_LOCAL, DIM], DT)
            nc.sync.dma_start(wo_sb[:], wo_d[:])

            # --- persistent activations (per batch for loose cross-phase deps) ---
            qT_b = [
                store.tile([DH, H_LOCAL, N], DT, tag=f"qT{b}", name=f"qT{b}")
                for b in range(B)
            ]
            kT_b = [
                store.tile([DH, N], DT, tag=f"kT{b}", name=f"kT{b}") for b in range(B)
            ]
            v_b = [
                store.tile([128, TT, DH], DT, tag=f"v{b}", name=f"v{b}")
                for b in range(B)
            ]
            aoT_b = [
                store.tile([DH, H_LOCAL, N], DT, tag=f"aoT{b}", name=f"aoT{b}")
                for b in range(B)
            ]

            ln_state = {}

            def ln_stats_a(b, cg):
                """First half of LN stats (tiles 0-1) — split so the DVE queue
                never sees one long stats lump that delays PSUM evictions."""
                xts = load_x(b, cg)
                mr = small.tile([128, 4, 2], F32, tag="mr")
                for t in (0, 1):
                    stats = small.tile([128, 4, 6], F32, tag="stats")
                    for i in range(4):
                        nc.vector.bn_stats(
                            out=stats[:, i, :], in_=xts[t][:, i * 512 : (i + 1) * 512]
                        )
                    nc.vector.bn_aggr(out=mr[:, t, :], in_=stats[:])
                ln_state[(b, cg)] = (xts, mr)

            def ln_stats_b(b, cg):
                """Second half: stats tiles 2-3, rsqrt, row bounce, broadcast.

                Returns (mu_bc, rstd_bc, m1_bc) row-broadcast views [128, CHUNK].
                """
                xts, mr = ln_state.pop((b, cg))
                for t in (2, 3):
                    stats = small.tile([128, 4, 6], F32, tag="stats")
                    for i in range(4):
                        nc.vector.bn_stats(
                            out=stats[:, i, :], in_=xts[t][:, i * 512 : (i + 1) * 512]
                        )
                    nc.vector.bn_aggr(out=mr[:, t, :], in_=stats[:])
                # Newton rsqrt: y = rsqrt(var+eps), y0 = 1.25-0.25v, 4 iters
                mp = small.tile([128, 2, 4], F32, tag="mp")
                ve = small.tile([128, 4], F32, tag="ve")
                nc.vector.tensor_scalar(
                    out=ve[:], in0=mr[:, :, 1], scalar1=EPS, scalar2=None,
                    op0=mybir.AluOpType.add,
                )
                y = small.tile([128, 4], F32, tag="y")
                nc.vector.tensor_scalar(
                    out=y[:], in0=ve[:], scalar1=-0.25, scalar2=1.25,
                    op0=mybir.AluOpType.mult, op1=mybir.AluOpType.add,
                )
                a = small.tile([128, 4], F32, tag="a")
                for it in range(4):
                    nc.vector.tensor_mul(a[:], y[:], y[:])
                    nc.vector.tensor_mul(a[:], a[:], ve[:])
                    nc.vector.tensor_scalar(
                        out=a[:], in0=a[:], scalar1=-0.5, scalar2=1.5,
                        op0=mybir.AluOpType.mult, op1=mybir.AluOpType.add,
                    )
                    dst = mp[:, 1, :] if it == 3 else y[:]
                    nc.vector.tensor_mul(dst, y[:], a[:])
                nc.vector.tensor_copy(mp[:, 0, :], mr[:, :, 0])
                # partition-layout -> row-layout via DRAM bounce (SWDGE):
                # mu stays a row (consumed by rank-1 correction matmuls on
                # the PE); only rstd needs a partition broadcast.
                md = drp.tile([128, 2, 4], F32, tag="md")
                nc.gpsimd.dma_start(md[:], mp[:])
                row2 = rwp.tile([1, 2 * CHUNK], DT, tag="row2", name=f"row2_{b}_{cg}")
                nc.gpsimd.dma_start(row2[:], md.rearrange("p c t -> c t p"))
                rsbc = bcp.tile([128, CHUNK], DT, tag="rsbc", name=f"rsbc_{b}_{cg}")
                nc.gpsimd.partition_broadcast(rsbc[:], row2[:, CHUNK : 2 * CHUNK])
                return row2, rsbc  # mu row [1,0:CHUNK] of row2, rstd bcast

            def rope_evict(dst, src_ps, cosp, sinp):
                # dst = ROPE(src) * rstd  (rstd folded into cosp/sinp;
                # the -cs*mu correction was accumulated on the PE)
                pc = ropep.tile([DH, CHUNK], DT, tag="pc")
                nc.vector.tensor_copy(pc[:], src_ps[:])
                rot = ropep.tile([DH, CHUNK], DT, tag="rot")
                nc.scalar.copy(rot[0:64, :], pc[64:128, :])
                nc.scalar.copy(rot[64:128, :], pc[0:64, :])
                tmp = ropep.tile([DH, CHUNK], DT, tag="tmp")
                nc.vector.tensor_mul(tmp[:], pc[:], cosp[:])
                nc.vector.tensor_mul(rot[:], rot[:], sinp[:])
                nc.vector.tensor_add(dst, tmp[:], rot[:])

            def chunk_proj(b, cg, lnr):
                c0 = cg * CHUNK
                row2, rstd_bc = lnr
                xtc = load_xt(b, cg)
                # rstd folded into the rope multipliers (shared by q0,q1,k)
                cosp = ropep.tile([DH, CHUNK], DT, tag="cosp")
                nc.vector.tensor_mul(cosp[:], cos_sb[:, c0 : c0 + CHUNK], rstd_bc)
                sinp = ropep.tile([DH, CHUNK], DT, tag="sinp")
                nc.vector.tensor_mul(sinp[:], sin_sb[:, c0 : c0 + CHUNK], rstd_bc)
                # wave 1: q heads
                qt0 = ps.tile([DH, CHUNK], F32, tag="acc", bufs=3)
                qt1 = ps.tile([DH, CHUNK], F32, tag="acc", bufs=3)
                for kt in range(KT):
                    rhs = xtc[:, kt, :]
                    nc.tensor.matmul(
                        qt0[:], wq_sb[:, kt, 0:128], rhs,
                        start=(kt == 0), stop=False,
                    )
                    nc.tensor.matmul(
                        qt1[:], wq_sb[:, kt, 128:256], rhs,
                        start=(kt == 0), stop=False,
                    )
                nc.tensor.matmul(
                    qt0[:], csr_sb[:, 0, :], row2[:, 0:CHUNK],
                    start=False, stop=True,
                )
                nc.tensor.matmul(
                    qt1[:], csr_sb[:, 1, :], row2[:, 0:CHUNK],
                    start=False, stop=True,
                )
                rope_evict(qT_b[b][:, 0, c0 : c0 + CHUNK], qt0, cosp, sinp)
                rope_evict(qT_b[b][:, 1, c0 : c0 + CHUNK], qt1, cosp, sinp)
                # wave 2: k, v
                ktp = ps.tile([DH, CHUNK], F32, tag="acc", bufs=3)
                vtp = ps.tile([DH, CHUNK], F32, tag="acc", bufs=3)
                for kt in range(KT):
                    rhs = xtc[:, kt, :]
                    nc.tensor.matmul(
                        ktp[:], wk_sb[:, kt, :], rhs,
                        start=(kt == 0), stop=False,
                    )
                    nc.tensor.matmul(
                        vtp[:], wv_sb[:, kt, :], rhs,
                        start=(kt == 0), stop=False,
                    )
                nc.tensor.matmul(
                    ktp[:], csr_sb[:, 2, :], row2[:, 0:CHUNK],
                    start=False, stop=True,
                )
                nc.tensor.matmul(
                    vtp[:], csr_sb[:, 3, :], row2[:, 0:CHUNK],
                    start=False, stop=True,
                )
                rope_evict(kT_b[b][:, c0 : c0 + CHUNK], ktp, cosp, sinp)
                # v: scale by rstd, then PE-transpose to natural [tok, dh]
                vT = vb.tile([DH, CHUNK], DT, tag="vT")
                nc.vector.tensor_mul(vT[:], vtp[:], rstd_bc)
                vn_ps = ps.tile([128, 512], F32, tag="s", bufs=2)
                for tv in range(4):
                    nc.tensor.matmul(
                        vn_ps[:, tv * 128 : (tv + 1) * 128],
                        vT[:, tv * 128 : (tv + 1) * 128],
                        ident[:],
                    )
                nc.scalar.copy(
                    v_b[b][:, cg * 4 : (cg + 1) * 4, :],
                    vn_ps[:].rearrange("p (t d) -> p t d", t=4),
                )

            def attn_group(b, h, qg):
                q0 = qg * QG
                nkt = (qg + 1) * (QG // 128)
                avT = ps.tile([DH, QG], F32, tag="av", bufs=2)
                sums = ps.tile([1, QG], F32, tag="sums", bufs=1)
                # software-pipelined emission: S(kt+1) is emitted BEFORE
                # AV(kt) so the PE FIFO never waits for exp(kt) — the exp
                # hides behind the next S matmul.
                pend = None
                for kt in range(nkt):
                    off = max(0, kt * 128 - q0)
                    diag = kt * 128 >= q0
                    st = ps.tile([128, QG], F32, tag="s", bufs=2)
                    nc.tensor.matmul(
                        st[:, off:],
                        kT_b[b][:, kt * 128 : (kt + 1) * 128],
                        qT_b[b][:, h, q0 + off : q0 + QG],
                        start=True,
                        stop=not diag,
                    )
                    if diag:  # causal mask accumulated on the PE itself
                        nc.tensor.matmul(
                            st[:, off : off + 128],
                            ident[:],
                            msk_sb[:],
                            start=False,
                            stop=True,
                        )
                    et = ep.tile([128, QG], DT, tag="et")
                    nc.scalar.activation(
                        out=et[:, off:],
                        in_=st[:, off:],
                        func=mybir.ActivationFunctionType.Exp,
                        scale=SCALE,
                    )
                    if pend is not None:
                        pkt, poff, pet = pend
                        nc.tensor.matmul(
                            avT[:, poff:],
                            v_b[b][:, pkt, :],
                            pet[:, poff:],
                            start=(pkt == 0),
                            stop=False,
                        )
                        nc.tensor.matmul(
                            sums[:, poff:],
                            ones_mm[:],
                            pet[:, poff:],
                            start=(pkt == 0),
                            stop=False,
                        )
                    pend = (kt, off, et)
                pkt, poff, pet = pend
                nc.tensor.matmul(
                    avT[:, poff:],
                    v_b[b][:, pkt, :],
                    pet[:, poff:],
                    start=(pkt == 0),
                    stop=True,
                )
                nc.tensor.matmul(
                    sums[:, poff:],
                    ones_mm[:],
                    pet[:, poff:],
                    start=(pkt == 0),
                    stop=True,
                )
                # reciprocal on a [128, 4] reshape via DRAM bounce
                # (DVE recip is FD-serial: [1,512] costs ~2.3us, [128,4] ~0.1us)
                ss = sm.tile([1, QG], F32, tag="ss")
                nc.vector.tensor_copy(ss[:], sums[:])
                dr = drp.tile([1, QG], F32, tag="dr")
                nc.gpsimd.dma_start(dr[:], ss[:])
                rt = sm.tile([128, 4], F32, tag="rt")
                nc.gpsimd.dma_start(
                    rt[:], dr.rearrange("o (p f) -> (o p) f", p=128)
                )
                nc.vector.reciprocal(out=rt[:], in_=rt[:])
                dr2 = drp.tile([1, QG], F32, tag="dr2")
                nc.gpsimd.dma_start(
                    dr2.rearrange("o (p f) -> (o p) f", p=128), rt[:]
                )
                rr = sm.tile([1, QG], F32, tag="rr")
                nc.gpsimd.dma_start(rr[:], dr2[:])
                rbc = sm.tile([128, QG], F32, tag="rbc")
                nc.gpsimd.partition_broadcast(rbc[:], rr[:])
                nc.vector.tensor_mul(
                    aoT_b[b][:, h, q0 : q0 + QG], avT[:], rbc[:]
                )

            def outproj_group(b, tt):
                ot = op.tile([128, DIM], DT, tag="ot")
                for dg in range(4):
                    opp = ps.tile([128, 512], F32, tag="acc", bufs=3)
                    for h in range(H_LOCAL):
                        nc.tensor.matmul(
                            opp[:],
                            aoT_b[b][:, h, tt * 128 : (tt + 1) * 128],
                            wo_sb[:, h, dg * 512 : (dg + 1) * 512],
                            start=(h == 0),
                            stop=(h == H_LOCAL - 1),
                        )
                    if dg % 2 == 0:
                        nc.scalar.copy(ot[:, dg * 512 : (dg + 1) * 512], opp[:])
                    else:
                        nc.vector.tensor_copy(ot[:, dg * 512 : (dg + 1) * 512], opp[:])
                nc.sync.dma_start(out_d[b, tt * 128 : (tt + 1) * 128, :], ot[:])

            for _rep in range(repeat):
                # LN stats run one full round ahead of the projection that
                # consumes them: the rank-1 correction matmul (and thus the
                # PSUM-group close + slot release) needs the mu row, so the
                # stats chain must never be the thing the PE waits on.
                lnr = {}
                ln_stats_a(0, 0)
                ln_stats_a(0, 1)
                lnr[(0, 0)] = ln_stats_b(0, 0)
                pre_xt[(0, 1)] = load_xt(0, 1)
                lnr[(0, 1)] = ln_stats_b(0, 1)
                chunk_proj(0, 0, lnr.pop((0, 0)))
                ln_stats_a(0, 2)
                lnr[(0, 2)] = ln_stats_b(0, 2)
                pre_xt[(0, 2)] = load_xt(0, 2)
                chunk_proj(0, 1, lnr.pop((0, 1)))
                ln_stats_a(0, 3)
                lnr[(0, 3)] = ln_stats_b(0, 3)
                pre_xt[(0, 3)] = load_xt(0, 3)
                chunk_proj(0, 2, lnr.pop((0, 2)))
                ln_stats_a(1, 0)
                lnr[(1, 0)] = ln_stats_b(1, 0)
                pre_xt[(1, 0)] = load_xt(1, 0)
                chunk_proj(0, 3, lnr.pop((0, 3)))
                # phase 2: b0 attention + b1 projections + b0 out-proj (lagged)
                for qg in range(NQG):
                    if qg < NQG - 1:
                        ln_stats_a(1, qg + 1)
                    if qg > 0:
                        for tt in range(4 * (qg - 1), 4 * qg):
                            outproj_group(0, tt)
                    attn_group(0, 0, qg)
                    if qg < NQG - 1:
                        lnr[(1, qg + 1)] = ln_stats_b(1, qg + 1)
                        pre_xt[(1, qg + 1)] = load_xt(1, qg + 1)
                    attn_group(0, 1, qg)
                    chunk_proj(1, qg, lnr.pop((1, qg)))
                # phase 3: b1 attention + remaining out-proj (still lagged)
                for qg in range(NQG):
                    if qg == 0:
                        for tt in range(12, 16):
                            outproj_group(0, tt)
                    else:
                        for tt in range(4 * (qg - 1), 4 * qg):
                            outproj_group(1, tt)
                    attn_group(1, 0, qg)
                    attn_group(1, 1, qg)
                for tt in range(12, 16):
                    outproj_group(1, tt)

    nc.compile()
    return nc


def make_in_maps(x, gamma, Wq, Wkv, Wo):
    xbf = np.asarray(x, dtype=np.float32).astype(DT_NP)
    x_nat = np.ascontiguousarray(xbf)
    # xT[p, b, cg, kt, t] = x[b, cg*CHUNK + t, kt*128 + p]
    xT = np.ascontiguousarray(
        xbf.reshape(B, NCH, CHUNK, KT, 128).transpose(4, 0, 1, 3, 2)
    )
    g = np.asarray(gamma, dtype=np.float32)
    Wq = np.asarray(Wq, dtype=np.float32) * g[:, None]
    Wkv = np.asarray(Wkv, dtype=np.float32) * g[:, None]
    Wo = np.asarray(Wo, dtype=np.float32)

    t = np.arange(N, dtype=np.float64)
    inv = 1.0 / (10000.0 ** (np.arange(0, DH, 2, dtype=np.float64) / DH))  # [64]
    fr = np.outer(inv, t)  # [d, t]
    cosT = np.concatenate([np.cos(fr), np.cos(fr)], 0).astype(DT_NP)
    sinT = np.concatenate([-np.sin(fr), np.sin(fr)], 0).astype(DT_NP)
    mask = np.where(
        np.arange(128)[:, None] > np.arange(128)[None, :], NEG, 0.0
    ).astype(DT_NP)

    def pt(w):  # [DIM, M] -> [128, KT, M] partition-major
        return np.ascontiguousarray(
            w.reshape(KT, 128, -1).transpose(1, 0, 2).astype(DT_NP)
        )

    Wk = Wkv[:, :DH]
    Wv = Wkv[:, DH:]
    # negated column sums for the rank-1 LN-fold correction matmul
    csk = Wk.astype(DT_NP).astype(np.float32).sum(0)
    csv = Wv.astype(DT_NP).astype(np.float32).sum(0)
    maps = []
    for c in range(N_CORES):
        Wq_c = Wq[:, c * H_LOCAL * DH : (c + 1) * H_LOCAL * DH]
        wq_c = pt(Wq_c)
        csq_c = Wq_c.astype(DT_NP).astype(np.float32).sum(0).reshape(H_LOCAL, DH)
        csr_c = np.ascontiguousarray(
            (-np.stack([csq_c[0], csq_c[1], csk, csv])[None]).astype(DT_NP)
        )
        wo_c = np.ascontiguousarray(
            Wo[c * H_LOCAL * DH : (c + 1) * H_LOCAL * DH]
            .reshape(H_LOCAL, DH, DIM)
            .transpose(1, 0, 2)
            .astype(DT_NP)
        )
        maps.append(
            {
                "x_in": x_nat,
                "xT": xT,
                "wq": wq_c,
                "wk": pt(Wk),
                "wv": pt(Wv),
                "wo": wo_c,
                "csr": csr_c,
                "cosT": cosT,
                "sinT": sinT,
                "mask": mask,
            }
        )
    return maps


_NC_CACHE = {}


def get_nc(repeat=1, phase=4):
    key = (repeat,)
    if key not in _NC_CACHE:
        _NC_CACHE[key] = build_nc(repeat)
    return _NC_CACHE[key]


def kernel(x, gamma, Wq, Wkv, Wo, _trace=False, _repeat=1):
    from concourse import bass_utils

    nc = get_nc(_repeat)
    in_maps = make_in_maps(x, gamma, Wq, Wkv, Wo)
    res = bass_utils.run_bass_kernel_spmd(
        nc, in_maps, core_ids=list(range(N_CORES)), trace=_trace
    )
    out = np.zeros((B, N, DIM), dtype=np.float32)
    for r in res.results:
        out += np.asarray(r["out_partial"], dtype=np.float32)
    if _trace:
        kernel.last_results = res
    return out
